# revision 19
# baseline (speedup 1.0000x reference)
"""Expert-parallel MoE MLP kernel for Trainium2 (8 NeuronCores).

Problem: x[B=2,S=1024,H=1024] f32, expert_indices[B,S] int, 16 experts,
gate/up_proj[E,H,I], down_proj[E,I,H] (H=I=1024):
    out[n] = silu(x_n @ Wg[e_n]) * (x_n @ Wu[e_n]) @ Wd[e_n].T

Sharding: expert parallelism — core c owns experts {2c, 2c+1}. The host
groups tokens by expert (the "all-to-all dispatch" runs on host since the
kernel contract is full-input -> full-output), pads each expert's token
block to a fixed capacity, and each core runs dense per-expert GEMMs.

Device layout (per core, per expert e) keeps features on partitions so no
on-chip transposes are needed:
    xt    = X_e^T                [H=1024, P]
    Gt[i,n] = sum_h Wg[h,i]*xt[h,n];  inter = silu(Gt)*Ut
    Out^T[j,n] = sum_k WdT[k,j]*inter[k,n]   (WdT = Wd.T, host-transposed)

Matmuls run as float32r (TF32-like, 10 explicit HW mantissa bits,
~1.5e-4 rel error) at 1 cycle/row for moving dim >=256; all operands are
pre-rounded (unrounded f32r operands hard-fault the exec unit).

Perf structure:
  - weights are 24 MB/core of mandatory HBM traffic (the roofline); they
    are host-packed partition-major so each 1 MB DMA chunk moves 8 KB
    contiguous per-partition runs (near line-rate)
  - expert-0 gate weights + xt are interleaved per-chunk at the head of
    the HWDGE FIFO, and phases accumulate h-outer into 8 PSUM banks, so
    the first matmul starts after ~1 MB instead of ~6 MB
  - token blocks are DMA'd at their real (padded-to-16) width; the matmul
    still streams 256 columns, with the pad region zeroed once on-chip
  - each expert's outputs are staged in SBUF and shipped as one DMA so
    the kernel tail isn't serialized on 8 small-DMA dispatches
"""

import math

import numpy as np

E = 16
H = 1024
HT = 8          # H / 128 partition tiles
HB = 2          # h-blocks per weight DMA chunk (1 MB chunks)
N_CORES = 8
EPC = E // N_CORES  # experts per core
CW = 256        # matmul moving-dim width (>=256 keeps f32r at 1 cyc/row)

_NC_CACHE = {}


def _round_f32r(a, mant=16):
    """Round-to-nearest to `mant` explicit mantissa bits (f32r operand prep)."""
    xi = np.ascontiguousarray(a, dtype=np.float32).view(np.uint32).astype(np.uint64)
    drop = 23 - mant
    half = np.uint64(1 << (drop - 1))
    mask = np.uint64((~((1 << drop) - 1)) & 0xFFFFFFFF)
    return ((xi + half) & mask).astype(np.uint32).view(np.float32)


def _build_nc(ch: int, pio: int, mmdt: str = "float32r"):
    """One SPMD program: EPC experts, ch chunks of CW token-slots per expert,
    pio real (DMA'd) token columns per expert, pio <= ch*CW."""
    import concourse.tile as tile
    from concourse import bacc, mybir
    from concourse.bass import ts

    f32 = mybir.dt.float32
    f32r = mybir.dt[mmdt]

    nc = bacc.Bacc("TRN2", target_bir_lowering=False, debug=False,
                   num_devices=N_CORES)
    # weights packed partition-major: w[e, proj, p, h, :] = Wproj[e][h*128+p, :]
    w = nc.dram_tensor("w", [EPC, 3, 128, HT, H], f32r, kind="ExternalInput")
    xt = nc.dram_tensor("xt", [EPC, 128, HT, pio], f32r, kind="ExternalInput")
    out = nc.dram_tensor("out", [EPC, 128, HT, pio], f32r, kind="ExternalOutput")

    assert ch == 1, "token groups above one chunk go through the round loop"
    with tile.TileContext(nc) as tc:
        with (
            tc.tile_pool(name="wp", bufs=1) as wp,
            tc.tile_pool(name="xp", bufs=1) as xp,
            tc.tile_pool(name="gp", bufs=2) as gp,
            tc.tile_pool(name="ip", bufs=2) as ip,
            tc.tile_pool(name="op", bufs=2) as op,
            tc.tile_pool(name="ps", bufs=8, space="PSUM") as ps,
        ):
            # all weights stay resident in SBUF (96 KB/partition at bf16):
            # every DMA is issued upfront in consumption order, so the HBM
            # stream never stalls on compute draining a recycled buffer.
            w_sb = wp.tile([128, EPC, 3, HT, H], f32r)
            x_sb = xp.tile([128, EPC, HT, pio], f32r)

            def wpart(e, proj, h0, h1, eng=None):
                (eng or nc.sync).dma_start(w_sb[:, e, proj, h0:h1, :],
                                           w[e, proj, :, h0:h1, :])

            # x on the scalar HWDGE queue (free early; dispatch overlaps the
            # weight dispatches on sync). Weights on sync with tiny pieces at
            # the head so the first matmul starts ASAP, then 2-h chunks that
            # pace the PE smoothly (1 chunk DMA ~= 16 matmuls of PE work).
            nc.scalar.dma_start(x_sb[:, 0, 0:1, :], xt[0, :, 0:1, :])
            nc.scalar.dma_start(x_sb[:, 0, 1:HT, :], xt[0, :, 1:HT, :])
            nc.scalar.dma_start(x_sb[:, 1, :, :], xt[1])
            # <=14 weight DMAs on sync: the framework cycles 8 completion
            # semaphores, so a deep queue of small chunks stalls dispatch
            # (dispatch k+8 waits for transfer k) and starves the engines
            wpart(0, 0, 0, 1)
            wpart(0, 0, 1, 2)
            wpart(0, 0, 2, 4)
            wpart(0, 0, 4, 8)
            for e in range(EPC):
                for proj in range(3):
                    if (e, proj) != (0, 0):
                        wpart(e, proj, 0, 4)
                        wpart(e, proj, 4, 8)



            for e in range(EPC):
                def wsl(proj, h, col, e=e):
                    return w_sb[:, e, proj, h, col]

                g_sb = gp.tile([128, HT, pio], f32)     # silu(Gt)
                i_sb = ip.tile([128, HT, pio], f32r)    # inter = silu(Gt)*Ut
                o_sb = op.tile([128, HT, pio], f32r, tag="o")
                # Each GEMM accumulates h-outer (chunks consumed as they
                # land) for h=0..HT-2, then finishes banks one at a time so
                # each bank's consumer fires immediately and frees the bank
                # instead of the whole chain serializing at the GEMM end.
                g_ps = [ps.tile([128, pio], f32, tag="ps", name=f"gps{i_}")
                        for i_ in range(HT)]
                for h in range(HT - 1):
                    for i in range(HT):
                        nc.tensor.matmul(
                            g_ps[i][:], wsl(0, h, ts(i, 128)),
                            x_sb[:, e, h, :],
                            start=(h == 0), stop=False)
                for i in range(HT):
                    nc.tensor.matmul(
                        g_ps[i][:], wsl(0, HT - 1, ts(i, 128)),
                        x_sb[:, e, HT - 1, :], start=False, stop=True)
                    nc.scalar.activation(
                        g_sb[:, i, :], g_ps[i][:],
                        mybir.ActivationFunctionType.Silu)
                # up
                u_ps = [ps.tile([128, pio], f32, tag="ps", name=f"ups{i_}")
                        for i_ in range(HT)]
                for h in range(HT - 1):
                    for i in range(HT):
                        nc.tensor.matmul(
                            u_ps[i][:], wsl(1, h, ts(i, 128)),
                            x_sb[:, e, h, :],
                            start=(h == 0), stop=False)
                for i in range(HT):
                    nc.tensor.matmul(
                        u_ps[i][:], wsl(1, HT - 1, ts(i, 128)),
                        x_sb[:, e, HT - 1, :], start=False, stop=True)
                    nc.vector.tensor_mul(
                        i_sb[:, i, :], g_sb[:, i, :], u_ps[i][:])
                # down: k-outer — matmuls for contraction step k wait only
                # on mul_k, so they stream with the down weight chunks
                o_ps = [ps.tile([128, pio], f32, tag="ps", name=f"ops{i_}")
                        for i_ in range(HT)]
                for k in range(HT - 1):
                    for j in range(HT):
                        nc.tensor.matmul(
                            o_ps[j][:], wsl(2, k, ts(j, 128)),
                            i_sb[:, k, :],
                            start=(k == 0), stop=False)
                # last contraction step per bank + cast + ship; casts split
                # across vector+scalar so the end chain halves
                for hf in range(2):
                    for q in range(2 * hf, 2 * hf + 2):
                        for j in (2 * q, 2 * q + 1):
                            nc.tensor.matmul(
                                o_ps[j][:], wsl(2, HT - 1, ts(j, 128)),
                                i_sb[:, HT - 1, :], start=False, stop=True)
                        nc.vector.tensor_copy(o_sb[:, 2 * q, :],
                                              o_ps[2 * q][:])
                        nc.scalar.copy(o_sb[:, 2 * q + 1, :],
                                       o_ps[2 * q + 1][:])
                    nc.sync.dma_start(out[e, :, 4 * hf:4 * hf + 4, :],
                                      o_sb[:, 4 * hf:4 * hf + 4, :])
    nc.compile()
    return nc


MM_DTYPE = "bfloat16"     # "float32r" (TF32, ~2.4e-4) or "bfloat16" (~2e-3, 2x DMA win)


def _get_nc(ch: int, pio: int):
    key = (ch, pio, MM_DTYPE)
    if key not in _NC_CACHE:
        _NC_CACHE[key] = _build_nc(ch, pio, MM_DTYPE)
    return _NC_CACHE[key]


_ROUND_CAP = 256          # max tokens/expert per device round (one chunk)


def _kernel_once(x, expert_indices, gate_proj, up_proj, down_proj):
    from concourse.bass_utils import run_bass_kernel_spmd

    x = np.ascontiguousarray(x, dtype=np.float32)
    gate_proj = np.ascontiguousarray(gate_proj, dtype=np.float32)
    up_proj = np.ascontiguousarray(up_proj, dtype=np.float32)
    down_proj = np.ascontiguousarray(down_proj, dtype=np.float32)
    b, s, h = x.shape
    assert (h, gate_proj.shape) == (H, (E, H, H)), (x.shape, gate_proj.shape)

    n = b * s
    xf = x.reshape(n, h)
    idx = np.asarray(expert_indices).reshape(n).astype(np.int64)

    order = np.argsort(idx, kind="stable")       # token ids grouped by expert
    counts = np.bincount(idx, minlength=E)
    starts = np.zeros(E + 1, dtype=np.int64)
    np.cumsum(counts, out=starts[1:])
    maxc = int(counts.max())
    ch = max(1, math.ceil(maxc / CW))
    pio = min(ch * CW, max(16, 2 * math.ceil(maxc / 2)))

    # per-core inputs; weights packed partition-major [EPC,3,128,HT,H]
    if MM_DTYPE == "bfloat16":
        import ml_dtypes
        def _prep(a):
            return np.ascontiguousarray(a, dtype=np.float32).astype(
                np.dtype(ml_dtypes.bfloat16))
    else:
        _prep = _round_f32r
    wr = _prep(
        np.stack([gate_proj, up_proj, down_proj.transpose(0, 2, 1)], axis=1)
    ).reshape(N_CORES, EPC, 3, HT, 128, H).transpose(0, 1, 2, 4, 3, 5)
    in_maps = []
    tok_ids = []
    for c in range(N_CORES):
        xt_c = np.zeros((EPC, H, pio), dtype=np.float32)
        toks = []
        for le in range(EPC):
            e = c * EPC + le
            te = order[starts[e]:starts[e + 1]]
            toks.append(te)
            xt_c[le, :, :len(te)] = xf[te].T
        tok_ids.append(toks)
        in_maps.append({
            "w": np.ascontiguousarray(wr[c]),
            "xt": _prep(xt_c).reshape(EPC, HT, 128, pio)
                  .transpose(0, 2, 1, 3).copy(),
        })

    nc = _get_nc(ch, pio)
    res = run_bass_kernel_spmd(nc, in_maps, core_ids=list(range(N_CORES)))

    out = np.empty((n, h), dtype=np.float32)
    for c in range(N_CORES):
        o = np.asarray(res.results[c]["out"]).astype(np.float32)
        for le in range(EPC):                    # o: [EPC, 128, HT, pio]
            te = tok_ids[c][le]
            oe = o[le].transpose(1, 0, 2).reshape(h, pio)   # [H, pio]
            out[te] = oe[:, :len(te)].T
    return out.reshape(b, s, h)


def kernel(x, expert_indices, gate_proj, up_proj, down_proj):
    """Full-input -> full-output entry point.

    Tokens-per-expert above _ROUND_CAP (pathological skew; SBUF bound)
    are handled by running the device kernel in multiple rounds over
    disjoint token slices — outputs are per-token independent."""
    idx = np.asarray(expert_indices)
    counts = np.bincount(idx.reshape(-1).astype(np.int64), minlength=E)
    if counts.max() <= _ROUND_CAP:
        return _kernel_once(x, expert_indices, gate_proj, up_proj, down_proj)

    b, s, h = x.shape
    n = b * s
    xf = np.ascontiguousarray(x, dtype=np.float32).reshape(n, h)
    idxf = idx.reshape(n).astype(np.int64)
    order = np.argsort(idxf, kind="stable")
    starts = np.zeros(E + 1, dtype=np.int64)
    np.cumsum(np.bincount(idxf, minlength=E), out=starts[1:])
    out = np.empty((n, h), dtype=np.float32)
    rounds = math.ceil(counts.max() / _ROUND_CAP)
    for r in range(rounds):
        sel = np.concatenate([
            order[starts[e] + r * _ROUND_CAP:
                  min(starts[e] + (r + 1) * _ROUND_CAP, starts[e + 1])]
            for e in range(E)])
        if not len(sel):
            continue
        xr = xf[sel].reshape(1, len(sel), h)
        ir = idxf[sel].reshape(1, len(sel))
        out[sel] = _kernel_once(
            xr, ir, gate_proj, up_proj, down_proj).reshape(len(sel), h)
    return out.reshape(b, s, h)



# revision 20
# speedup vs baseline: 1.1206x; 1.1206x over previous
"""Expert-parallel MoE MLP kernel for Trainium2 (8 NeuronCores).

Problem: x[B=2,S=1024,H=1024] f32, expert_indices[B,S] int, 16 experts,
gate/up_proj[E,H,I], down_proj[E,I,H] (H=I=1024):
    out[n] = silu(x_n @ Wg[e_n]) * (x_n @ Wu[e_n]) @ Wd[e_n].T

Sharding: expert parallelism — core c owns experts {2c, 2c+1}. The host
groups tokens by expert (the "all-to-all dispatch" runs on host since the
kernel contract is full-input -> full-output), pads each expert's token
block to a fixed capacity, and each core runs dense per-expert GEMMs.

Device layout (per core, per expert e) keeps features on partitions so no
on-chip transposes are needed:
    xt    = X_e^T                [H=1024, P]
    Gt[i,n] = sum_h Wg[h,i]*xt[h,n];  inter = silu(Gt)*Ut
    Out^T[j,n] = sum_k WdT[k,j]*inter[k,n]   (WdT = Wd.T, host-transposed)

Matmuls run as float32r (TF32-like, 10 explicit HW mantissa bits,
~1.5e-4 rel error) at 1 cycle/row for moving dim >=256; all operands are
pre-rounded (unrounded f32r operands hard-fault the exec unit).

Perf structure:
  - weights are 24 MB/core of mandatory HBM traffic (the roofline); they
    are host-packed partition-major so each 1 MB DMA chunk moves 8 KB
    contiguous per-partition runs (near line-rate)
  - expert-0 gate weights + xt are interleaved per-chunk at the head of
    the HWDGE FIFO, and phases accumulate h-outer into 8 PSUM banks, so
    the first matmul starts after ~1 MB instead of ~6 MB
  - token blocks are DMA'd at their real (padded-to-16) width; the matmul
    still streams 256 columns, with the pad region zeroed once on-chip
  - each expert's outputs are staged in SBUF and shipped as one DMA so
    the kernel tail isn't serialized on 8 small-DMA dispatches
"""

import math

import numpy as np

E = 16
H = 1024
HT = 8          # H / 128 partition tiles
HB = 2          # h-blocks per weight DMA chunk (1 MB chunks)
N_CORES = 8
EPC = E // N_CORES  # experts per core
CW = 256        # matmul moving-dim width (>=256 keeps f32r at 1 cyc/row)

_NC_CACHE = {}


def _round_f32r(a, mant=16):
    """Round-to-nearest to `mant` explicit mantissa bits (f32r operand prep)."""
    xi = np.ascontiguousarray(a, dtype=np.float32).view(np.uint32).astype(np.uint64)
    drop = 23 - mant
    half = np.uint64(1 << (drop - 1))
    mask = np.uint64((~((1 << drop) - 1)) & 0xFFFFFFFF)
    return ((xi + half) & mask).astype(np.uint32).view(np.float32)


def _build_nc(ch: int, pio: int, mmdt: str = "float32r"):
    """One SPMD program: EPC experts, ch chunks of CW token-slots per expert,
    pio real (DMA'd) token columns per expert, pio <= ch*CW."""
    import concourse.tile as tile
    from concourse import bacc, mybir
    from concourse.bass import ts

    f32 = mybir.dt.float32
    f32r = mybir.dt[mmdt]

    nc = bacc.Bacc("TRN2", target_bir_lowering=False, debug=False,
                   num_devices=N_CORES)
    # weights packed partition-major: w[e, proj, p, h, :] = Wproj[e][h*128+p, :]
    w = nc.dram_tensor("w", [EPC, 3, 128, HT, H], f32r, kind="ExternalInput")
    xt = nc.dram_tensor("xt", [EPC, 128, HT, pio], f32r, kind="ExternalInput")
    out = nc.dram_tensor("out", [EPC, 128, HT, pio], f32r, kind="ExternalOutput")

    assert ch == 1, "token groups above one chunk go through the round loop"
    with tile.TileContext(nc) as tc:
        with (
            tc.tile_pool(name="wp", bufs=1) as wp,
            tc.tile_pool(name="xp", bufs=1) as xp,
            tc.tile_pool(name="gp", bufs=2) as gp,
            tc.tile_pool(name="ip", bufs=2) as ip,
            tc.tile_pool(name="op", bufs=2) as op,
            tc.tile_pool(name="ps", bufs=8, space="PSUM") as ps,
        ):
            # all weights stay resident in SBUF (96 KB/partition at bf16):
            # every DMA is issued upfront in consumption order, so the HBM
            # stream never stalls on compute draining a recycled buffer.
            w_sb = wp.tile([128, EPC, 3, HT, H], f32r)
            x_sb = xp.tile([128, EPC, HT, pio], f32r)

            def wpart(e, proj, h0, h1, eng=None):
                (eng or nc.sync).dma_start(w_sb[:, e, proj, h0:h1, :],
                                           w[e, proj, :, h0:h1, :])

            # x on the scalar HWDGE queue (free early; dispatch overlaps the
            # weight dispatches on sync). Weights on sync with tiny pieces at
            # the head so the first matmul starts ASAP, then 2-h chunks that
            # pace the PE smoothly (1 chunk DMA ~= 16 matmuls of PE work).
            nc.scalar.dma_start(x_sb[:, 0, 0:1, :], xt[0, :, 0:1, :])
            nc.scalar.dma_start(x_sb[:, 0, 1:HT, :], xt[0, :, 1:HT, :])
            nc.scalar.dma_start(x_sb[:, 1, :, :], xt[1])
            # <=14 weight DMAs on sync: the framework cycles 8 completion
            # semaphores, so a deep queue of small chunks stalls dispatch
            # (dispatch k+8 waits for transfer k) and starves the engines
            wpart(0, 0, 0, 1)
            wpart(0, 0, 1, 2)
            wpart(0, 0, 2, 4)
            wpart(0, 0, 4, 8)
            for e in range(EPC):
                for proj in range(3):
                    if (e, proj) != (0, 0):
                        wpart(e, proj, 0, 4)
                        wpart(e, proj, 4, 8)



            for e in range(EPC):
                def wsl(proj, h, col, e=e):
                    return w_sb[:, e, proj, h, col]

                g_sb = gp.tile([128, HT, pio], f32)     # silu(Gt)
                i_sb = ip.tile([128, HT, pio], f32r)    # inter = silu(Gt)*Ut
                o_sb = op.tile([128, HT, pio], f32r, tag="o")
                # Each GEMM accumulates h-outer (chunks consumed as they
                # land) for h=0..HT-2, then finishes banks one at a time so
                # each bank's consumer fires immediately and frees the bank
                # instead of the whole chain serializing at the GEMM end.
                g_ps = [ps.tile([128, pio], f32, tag="ps", name=f"gps{i_}")
                        for i_ in range(HT)]
                for h in range(HT - 1):
                    for i in range(HT):
                        nc.tensor.matmul(
                            g_ps[i][:], wsl(0, h, ts(i, 128)),
                            x_sb[:, e, h, :],
                            start=(h == 0), stop=False)
                for i in range(HT):
                    nc.tensor.matmul(
                        g_ps[i][:], wsl(0, HT - 1, ts(i, 128)),
                        x_sb[:, e, HT - 1, :], start=False, stop=True)
                    nc.scalar.activation(
                        g_sb[:, i, :], g_ps[i][:],
                        mybir.ActivationFunctionType.Silu)
                # up
                u_ps = [ps.tile([128, pio], f32, tag="ps", name=f"ups{i_}")
                        for i_ in range(HT)]
                for h in range(HT - 1):
                    for i in range(HT):
                        nc.tensor.matmul(
                            u_ps[i][:], wsl(1, h, ts(i, 128)),
                            x_sb[:, e, h, :],
                            start=(h == 0), stop=False)
                for i in range(HT):
                    nc.tensor.matmul(
                        u_ps[i][:], wsl(1, HT - 1, ts(i, 128)),
                        x_sb[:, e, HT - 1, :], start=False, stop=True)
                    nc.vector.tensor_mul(
                        i_sb[:, i, :], g_sb[:, i, :], u_ps[i][:])
                # down: k-outer — matmuls for contraction step k wait only
                # on mul_k, so they stream with the down weight chunks
                o_ps = [ps.tile([128, pio], f32, tag="ps", name=f"ops{i_}")
                        for i_ in range(HT)]
                for k in range(HT - 1):
                    for j in range(HT):
                        nc.tensor.matmul(
                            o_ps[j][:], wsl(2, k, ts(j, 128)),
                            i_sb[:, k, :],
                            start=(k == 0), stop=False)
                # last contraction step per bank + cast + ship; casts split
                # across vector+scalar so the end chain halves
                for hf in range(2):
                    for q in range(2 * hf, 2 * hf + 2):
                        for j in (2 * q, 2 * q + 1):
                            nc.tensor.matmul(
                                o_ps[j][:], wsl(2, HT - 1, ts(j, 128)),
                                i_sb[:, HT - 1, :], start=False, stop=True)
                        nc.vector.tensor_copy(o_sb[:, 2 * q, :],
                                              o_ps[2 * q][:])
                        nc.scalar.copy(o_sb[:, 2 * q + 1, :],
                                       o_ps[2 * q + 1][:])
                    nc.sync.dma_start(out[e, :, 4 * hf:4 * hf + 4, :],
                                      o_sb[:, 4 * hf:4 * hf + 4, :])
    nc.compile()
    return nc


MM_DTYPE = "bfloat16"     # "float32r" (TF32, ~2.4e-4) or "bfloat16" (~2e-3, 2x DMA win)


def _get_nc(ch: int, pio: int):
    key = (ch, pio, MM_DTYPE)
    if key not in _NC_CACHE:
        _NC_CACHE[key] = _build_nc(ch, pio, MM_DTYPE)
    return _NC_CACHE[key]


_ROUND_CAP = 256          # max tokens/expert per device round (one chunk)


def _kernel_once(x, expert_indices, gate_proj, up_proj, down_proj):
    from concourse.bass_utils import run_bass_kernel_spmd

    x = np.ascontiguousarray(x, dtype=np.float32)
    gate_proj = np.ascontiguousarray(gate_proj, dtype=np.float32)
    up_proj = np.ascontiguousarray(up_proj, dtype=np.float32)
    down_proj = np.ascontiguousarray(down_proj, dtype=np.float32)
    b, s, h = x.shape
    assert (h, gate_proj.shape) == (H, (E, H, H)), (x.shape, gate_proj.shape)

    n = b * s
    xf = x.reshape(n, h)
    idx = np.asarray(expert_indices).reshape(n).astype(np.int64)

    order = np.argsort(idx, kind="stable")       # token ids grouped by expert
    counts = np.bincount(idx, minlength=E)
    starts = np.zeros(E + 1, dtype=np.int64)
    np.cumsum(counts, out=starts[1:])
    maxc = int(counts.max())
    ch = max(1, math.ceil(maxc / CW))
    pio = min(ch * CW, max(16, 16 * math.ceil(maxc / 16)))

    # per-core inputs; weights packed partition-major [EPC,3,128,HT,H]
    if MM_DTYPE == "bfloat16":
        import ml_dtypes
        def _prep(a):
            return np.ascontiguousarray(a, dtype=np.float32).astype(
                np.dtype(ml_dtypes.bfloat16))
    else:
        _prep = _round_f32r
    wr = _prep(
        np.stack([gate_proj, up_proj, down_proj.transpose(0, 2, 1)], axis=1)
    ).reshape(N_CORES, EPC, 3, HT, 128, H).transpose(0, 1, 2, 4, 3, 5)
    in_maps = []
    tok_ids = []
    for c in range(N_CORES):
        xt_c = np.zeros((EPC, H, pio), dtype=np.float32)
        toks = []
        for le in range(EPC):
            e = c * EPC + le
            te = order[starts[e]:starts[e + 1]]
            toks.append(te)
            xt_c[le, :, :len(te)] = xf[te].T
        tok_ids.append(toks)
        in_maps.append({
            "w": np.ascontiguousarray(wr[c]),
            "xt": _prep(xt_c).reshape(EPC, HT, 128, pio)
                  .transpose(0, 2, 1, 3).copy(),
        })

    nc = _get_nc(ch, pio)
    res = run_bass_kernel_spmd(nc, in_maps, core_ids=list(range(N_CORES)))

    out = np.empty((n, h), dtype=np.float32)
    for c in range(N_CORES):
        o = np.asarray(res.results[c]["out"]).astype(np.float32)
        for le in range(EPC):                    # o: [EPC, 128, HT, pio]
            te = tok_ids[c][le]
            oe = o[le].transpose(1, 0, 2).reshape(h, pio)   # [H, pio]
            out[te] = oe[:, :len(te)].T
    return out.reshape(b, s, h)


def kernel(x, expert_indices, gate_proj, up_proj, down_proj):
    """Full-input -> full-output entry point.

    Tokens-per-expert above _ROUND_CAP (pathological skew; SBUF bound)
    are handled by running the device kernel in multiple rounds over
    disjoint token slices — outputs are per-token independent."""
    idx = np.asarray(expert_indices)
    counts = np.bincount(idx.reshape(-1).astype(np.int64), minlength=E)
    if counts.max() <= _ROUND_CAP:
        return _kernel_once(x, expert_indices, gate_proj, up_proj, down_proj)

    b, s, h = x.shape
    n = b * s
    xf = np.ascontiguousarray(x, dtype=np.float32).reshape(n, h)
    idxf = idx.reshape(n).astype(np.int64)
    order = np.argsort(idxf, kind="stable")
    starts = np.zeros(E + 1, dtype=np.int64)
    np.cumsum(np.bincount(idxf, minlength=E), out=starts[1:])
    out = np.empty((n, h), dtype=np.float32)
    rounds = math.ceil(counts.max() / _ROUND_CAP)
    for r in range(rounds):
        sel = np.concatenate([
            order[starts[e] + r * _ROUND_CAP:
                  min(starts[e] + (r + 1) * _ROUND_CAP, starts[e + 1])]
            for e in range(E)])
        if not len(sel):
            continue
        xr = xf[sel].reshape(1, len(sel), h)
        ir = idxf[sel].reshape(1, len(sel))
        out[sel] = _kernel_once(
            xr, ir, gate_proj, up_proj, down_proj).reshape(len(sel), h)
    return out.reshape(b, s, h)



# revision 22
# speedup vs baseline: 1.1235x; 1.0026x over previous
"""Expert-parallel MoE MLP kernel for Trainium2 (8 NeuronCores).

Problem: x[B=2,S=1024,H=1024] f32, expert_indices[B,S] int, 16 experts,
gate/up_proj[E,H,I], down_proj[E,I,H] (H=I=1024):
    out[n] = silu(x_n @ Wg[e_n]) * (x_n @ Wu[e_n]) @ Wd[e_n].T

Sharding: expert parallelism — core c owns experts {2c, 2c+1}. The host
groups tokens by expert (the "all-to-all dispatch" runs on host since the
kernel contract is full-input -> full-output), pads each expert's token
block to a fixed capacity, and each core runs dense per-expert GEMMs.

Device layout (per core, per expert e) keeps features on partitions so no
on-chip transposes are needed:
    xt    = X_e^T                [H=1024, P]
    Gt[i,n] = sum_h Wg[h,i]*xt[h,n];  inter = silu(Gt)*Ut
    Out^T[j,n] = sum_k WdT[k,j]*inter[k,n]   (WdT = Wd.T, host-transposed)

Matmuls run as float32r (TF32-like, 10 explicit HW mantissa bits,
~1.5e-4 rel error) at 1 cycle/row for moving dim >=256; all operands are
pre-rounded (unrounded f32r operands hard-fault the exec unit).

Perf structure:
  - weights are 24 MB/core of mandatory HBM traffic (the roofline); they
    are host-packed partition-major so each 1 MB DMA chunk moves 8 KB
    contiguous per-partition runs (near line-rate)
  - expert-0 gate weights + xt are interleaved per-chunk at the head of
    the HWDGE FIFO, and phases accumulate h-outer into 8 PSUM banks, so
    the first matmul starts after ~1 MB instead of ~6 MB
  - token blocks are DMA'd at their real (padded-to-16) width; the matmul
    still streams 256 columns, with the pad region zeroed once on-chip
  - each expert's outputs are staged in SBUF and shipped as one DMA so
    the kernel tail isn't serialized on 8 small-DMA dispatches
"""

import math

import numpy as np

E = 16
H = 1024
HT = 8          # H / 128 partition tiles
HB = 2          # h-blocks per weight DMA chunk (1 MB chunks)
N_CORES = 8
EPC = E // N_CORES  # experts per core
CW = 256        # matmul moving-dim width (>=256 keeps f32r at 1 cyc/row)

_NC_CACHE = {}


def _round_f32r(a, mant=16):
    """Round-to-nearest to `mant` explicit mantissa bits (f32r operand prep)."""
    xi = np.ascontiguousarray(a, dtype=np.float32).view(np.uint32).astype(np.uint64)
    drop = 23 - mant
    half = np.uint64(1 << (drop - 1))
    mask = np.uint64((~((1 << drop) - 1)) & 0xFFFFFFFF)
    return ((xi + half) & mask).astype(np.uint32).view(np.float32)


def _build_nc(ch: int, pio: int, mmdt: str = "float32r"):
    """One SPMD program: EPC experts, ch chunks of CW token-slots per expert,
    pio real (DMA'd) token columns per expert, pio <= ch*CW."""
    import concourse.tile as tile
    from concourse import bacc, mybir
    from concourse.bass import ts

    f32 = mybir.dt.float32
    f32r = mybir.dt[mmdt]

    nc = bacc.Bacc("TRN2", target_bir_lowering=False, debug=False,
                   num_devices=N_CORES)
    # weights packed partition-major: w[e, proj, p, h, :] = Wproj[e][h*128+p, :]
    w = nc.dram_tensor("w", [EPC, 3, 128, HT, H], f32r, kind="ExternalInput")
    xt = nc.dram_tensor("xt", [EPC, 128, HT, pio], f32r, kind="ExternalInput")
    out = nc.dram_tensor("out", [EPC, 128, HT, pio], f32r, kind="ExternalOutput")

    assert ch == 1, "token groups above one chunk go through the round loop"
    with tile.TileContext(nc) as tc:
        with (
            tc.tile_pool(name="wp", bufs=1) as wp,
            tc.tile_pool(name="xp", bufs=1) as xp,
            tc.tile_pool(name="gp", bufs=2) as gp,
            tc.tile_pool(name="ip", bufs=2) as ip,
            tc.tile_pool(name="op", bufs=2) as op,
            tc.tile_pool(name="ps", bufs=8, space="PSUM") as ps,
        ):
            # all weights stay resident in SBUF (96 KB/partition at bf16):
            # every DMA is issued upfront in consumption order, so the HBM
            # stream never stalls on compute draining a recycled buffer.
            w_sb = wp.tile([128, EPC, 3, HT, H], f32r)
            x_sb = xp.tile([128, EPC, HT, pio], f32r)

            def wpart(e, proj, h0, h1, eng=None):
                (eng or nc.sync).dma_start(w_sb[:, e, proj, h0:h1, :],
                                           w[e, proj, :, h0:h1, :])

            # x on the scalar HWDGE queue (free early; dispatch overlaps the
            # weight dispatches on sync). Weights on sync with tiny pieces at
            # the head so the first matmul starts ASAP, then 2-h chunks that
            # pace the PE smoothly (1 chunk DMA ~= 16 matmuls of PE work).
            nc.scalar.dma_start(x_sb[:, 0, 0:1, :], xt[0, :, 0:1, :])
            nc.scalar.dma_start(x_sb[:, 0, 1:HT, :], xt[0, :, 1:HT, :])
            nc.scalar.dma_start(x_sb[:, 1, :, :], xt[1])
            # <=14 weight DMAs on sync: the framework cycles 8 completion
            # semaphores, so a deep queue of small chunks stalls dispatch
            # (dispatch k+8 waits for transfer k) and starves the engines
            wpart(0, 0, 0, 1)
            wpart(0, 0, 1, 2)
            wpart(0, 0, 2, 4)
            wpart(0, 0, 4, 8)
            for e in range(EPC):
                for proj in range(3):
                    if (e, proj) != (0, 0):
                        wpart(e, proj, 0, 4)
                        wpart(e, proj, 4, 8)



            for e in range(EPC):
                def wsl(proj, h, col, e=e):
                    return w_sb[:, e, proj, h, col]

                g_sb = gp.tile([128, HT, pio], f32)     # silu(Gt)
                i_sb = ip.tile([128, HT, pio], f32r)    # inter = silu(Gt)*Ut
                o_sb = op.tile([128, HT, pio], f32r, tag="o")
                # Each GEMM accumulates h-outer (chunks consumed as they
                # land) for h=0..HT-2, then finishes banks one at a time so
                # each bank's consumer fires immediately and frees the bank
                # instead of the whole chain serializing at the GEMM end.
                g_ps = [ps.tile([128, pio], f32, tag="ps", name=f"gps{i_}")
                        for i_ in range(HT)]
                for h in range(HT - 1):
                    for i in range(HT):
                        nc.tensor.matmul(
                            g_ps[i][:], wsl(0, h, ts(i, 128)),
                            x_sb[:, e, h, :],
                            start=(h == 0), stop=False)
                for i in range(HT):
                    nc.tensor.matmul(
                        g_ps[i][:], wsl(0, HT - 1, ts(i, 128)),
                        x_sb[:, e, HT - 1, :], start=False, stop=True)
                    nc.scalar.activation(
                        g_sb[:, i, :], g_ps[i][:],
                        mybir.ActivationFunctionType.Silu)
                # up
                u_ps = [ps.tile([128, pio], f32, tag="ps", name=f"ups{i_}")
                        for i_ in range(HT)]
                for h in range(HT - 1):
                    for i in range(HT):
                        nc.tensor.matmul(
                            u_ps[i][:], wsl(1, h, ts(i, 128)),
                            x_sb[:, e, h, :],
                            start=(h == 0), stop=False)
                for i in range(HT):
                    nc.tensor.matmul(
                        u_ps[i][:], wsl(1, HT - 1, ts(i, 128)),
                        x_sb[:, e, HT - 1, :], start=False, stop=True)
                    nc.vector.tensor_mul(
                        i_sb[:, i, :], g_sb[:, i, :], u_ps[i][:])
                # down: k-outer — matmuls for contraction step k wait only
                # on mul_k, so they stream with the down weight chunks
                o_ps = [ps.tile([128, pio], f32, tag="ps", name=f"ops{i_}")
                        for i_ in range(HT)]
                for k in range(HT - 1):
                    for j in range(HT):
                        nc.tensor.matmul(
                            o_ps[j][:], wsl(2, k, ts(j, 128)),
                            i_sb[:, k, :],
                            start=(k == 0), stop=False)
                # last contraction step per bank + cast + ship; casts split
                # across vector+scalar so the end chain halves
                for hf in range(2):
                    for q in range(2 * hf, 2 * hf + 2):
                        for j in (2 * q, 2 * q + 1):
                            nc.tensor.matmul(
                                o_ps[j][:], wsl(2, HT - 1, ts(j, 128)),
                                i_sb[:, HT - 1, :], start=False, stop=True)
                        nc.vector.tensor_copy(o_sb[:, 2 * q, :],
                                              o_ps[2 * q][:])
                        nc.scalar.copy(o_sb[:, 2 * q + 1, :],
                                       o_ps[2 * q + 1][:])
                    nc.sync.dma_start(out[e, :, 4 * hf:4 * hf + 4, :],
                                      o_sb[:, 4 * hf:4 * hf + 4, :])
    nc.compile()
    return nc


MM_DTYPE = "bfloat16"     # "float32r" (TF32, ~2.4e-4) or "bfloat16" (~2e-3, 2x DMA win)


def _get_nc(ch: int, pio: int):
    key = (ch, pio, MM_DTYPE)
    if key not in _NC_CACHE:
        _NC_CACHE[key] = _build_nc(ch, pio, MM_DTYPE)
    return _NC_CACHE[key]


_ROUND_CAP = 256          # max tokens/expert per device round (one chunk)


def _kernel_once(x, expert_indices, gate_proj, up_proj, down_proj):
    from concourse.bass_utils import run_bass_kernel_spmd

    x = np.ascontiguousarray(x, dtype=np.float32)
    gate_proj = np.ascontiguousarray(gate_proj, dtype=np.float32)
    up_proj = np.ascontiguousarray(up_proj, dtype=np.float32)
    down_proj = np.ascontiguousarray(down_proj, dtype=np.float32)
    b, s, h = x.shape
    assert (h, gate_proj.shape) == (H, (E, H, H)), (x.shape, gate_proj.shape)

    n = b * s
    xf = x.reshape(n, h)
    idx = np.asarray(expert_indices).reshape(n).astype(np.int64)

    order = np.argsort(idx, kind="stable")       # token ids grouped by expert
    counts = np.bincount(idx, minlength=E)
    starts = np.zeros(E + 1, dtype=np.int64)
    np.cumsum(counts, out=starts[1:])
    maxc = int(counts.max())
    ch = max(1, math.ceil(maxc / CW))
    pio = min(ch * CW, max(16, 16 * math.ceil(maxc / 16)))

    # per-core inputs; weights packed partition-major [EPC,3,128,HT,H]
    if MM_DTYPE == "bfloat16":
        import ml_dtypes
        def _prep(a):
            return np.ascontiguousarray(a, dtype=np.float32).astype(
                np.dtype(ml_dtypes.bfloat16))
    else:
        _prep = _round_f32r
    wr = _prep(
        np.stack([gate_proj, up_proj, down_proj.transpose(0, 2, 1)], axis=1)
    ).reshape(N_CORES, EPC, 3, HT, 128, H).transpose(0, 1, 2, 4, 3, 5)
    in_maps = []
    tok_ids = []
    for c in range(N_CORES):
        xt_c = np.zeros((EPC, H, pio), dtype=np.float32)
        toks = []
        for le in range(EPC):
            e = c * EPC + le
            te = order[starts[e]:starts[e + 1]]
            toks.append(te)
            xt_c[le, :, :len(te)] = xf[te].T
        tok_ids.append(toks)
        in_maps.append({
            "w": np.ascontiguousarray(wr[c]),
            "xt": _prep(xt_c).reshape(EPC, HT, 128, pio)
                  .transpose(0, 2, 1, 3).copy(),
        })

    nc = _get_nc(ch, pio)
    res = run_bass_kernel_spmd(nc, in_maps, core_ids=list(range(N_CORES)))

    out = np.empty((n, h), dtype=np.float32)
    for c in range(N_CORES):
        o = np.asarray(res.results[c]["out"]).astype(np.float32)
        for le in range(EPC):                    # o: [EPC, 128, HT, pio]
            te = tok_ids[c][le]
            oe = o[le].transpose(1, 0, 2).reshape(h, pio)   # [H, pio]
            out[te] = oe[:, :len(te)].T
    return out.reshape(b, s, h)


def kernel(x, expert_indices, gate_proj, up_proj, down_proj):
    """Full-input -> full-output entry point.

    Tokens-per-expert above _ROUND_CAP (pathological skew; SBUF bound)
    are handled by running the device kernel in multiple rounds over
    disjoint token slices — outputs are per-token independent."""
    idx = np.asarray(expert_indices)
    counts = np.bincount(idx.reshape(-1).astype(np.int64), minlength=E)
    if counts.max() <= _ROUND_CAP:
        return _kernel_once(x, expert_indices, gate_proj, up_proj, down_proj)

    b, s, h = x.shape
    n = b * s
    xf = np.ascontiguousarray(x, dtype=np.float32).reshape(n, h)
    idxf = idx.reshape(n).astype(np.int64)
    order = np.argsort(idxf, kind="stable")
    starts = np.zeros(E + 1, dtype=np.int64)
    np.cumsum(np.bincount(idxf, minlength=E), out=starts[1:])
    out = np.empty((n, h), dtype=np.float32)
    rounds = math.ceil(counts.max() / _ROUND_CAP)
    for r in range(rounds):
        sel = np.concatenate([
            order[starts[e] + r * _ROUND_CAP:
                  min(starts[e] + (r + 1) * _ROUND_CAP, starts[e + 1])]
            for e in range(E)])
        if not len(sel):
            continue
        xr = xf[sel].reshape(1, len(sel), h)
        ir = idxf[sel].reshape(1, len(sel))
        out[sel] = _kernel_once(
            xr, ir, gate_proj, up_proj, down_proj).reshape(len(sel), h)
    return out.reshape(b, s, h)

